# revision 41
# baseline (speedup 1.0000x reference)
"""GCN (7-layer, PyG-style symmetric-normalized message passing) on 8 TRN2
NeuronCores via Bass/Tile.

Strategy (graph/data parallel over nodes):
  - Nodes are assigned to 8 cores x 49 blocks of 128 slots each (load-balanced
    by in-degree so per-block message counts are nearly equal).
  - Per layer, per core:
      stage A: Z'' = dis * (h @ W) for the core's 6272 node slots (PE matmul
               per 128-node block + per-partition scale), node-major in SBUF.
      AllGather: bf16 Z'' shards -> full 50176-row table in local HBM.
      stage B: per dst block, gather Z''[src] rows for the block's edges
               (SWDGE dma_gather, int16 indices, table split in two halves to
               fit int16 range), build one-hot selector S on DVE
               (S[msg, slot] = (iota == segid)), and segment-sum on PE:
               O[feat, slot] += M_chunk.T @ S_chunk, with the self-loop chunk
               done as zbuf_block.T @ I directly from SBUF.
      epilogue: h' = relu(O * dis[dst] + b)  (DVE mult + ACT relu w/ bias).
  - Final: out = lin_w.T @ h7 + lin_b, one row per core, host reassembles.

All index/normalization prep is host-side numpy (graph routing), baked into
per-core input tensors; the float pipeline runs on device.
"""

import math
import os
import sys
from dataclasses import dataclass

import numpy as np

sys.path.insert(0, "/opt/trn_rl_repo")

import ml_dtypes  # noqa: E402

BF16 = ml_dtypes.bfloat16


def _env(name, default):
    return int(os.environ.get(name, default))


@dataclass
class GCNConfig:
    n_nodes: int = 50000
    n_edges: int = 600000
    feat: int = 128
    n_layers: int = 7
    n_cores: int = 8
    half: int = 32768  # int16 addressable rows per gather table half
    max_gather_chunks: int = _env("GCN_MAXCH", 32)  # max 128-idx chunks per dma_gather call
    n_swdge_queues: int = _env("GCN_NSWQ", 1)  # parallel SWDGE desc-gen queues (1..4)
    neg_pad: bool = False  # pad gather idx lists with -1 (skipped) vs 0
    ag_splits: int = _env("GCN_AGSPL", 1)  # split the per-layer AllGather into this many pieces
    balance_iters: int = _env("GCN_BAL", 0)  # lo/hi-aware node assignment refinement passes


def _plan(cfg: GCNConfig, edge_index: np.ndarray):
    """Host graph prep: node->(core,block,slot) assignment, per-block sorted
    edge lists split by table half, padding, and all static counts."""
    import heapq

    N, C = cfg.n_nodes, cfg.n_cores
    nloc = N // C
    nb = (nloc + 127) // 128
    nlocp = nb * 128
    ntab = nlocp * C
    nblocks = C * nb

    src = np.asarray(edge_index[0], dtype=np.int64)
    dst = np.asarray(edge_index[1], dtype=np.int64)
    deg = np.bincount(dst, minlength=N).astype(np.int64) + 1  # + self loop
    dis = (1.0 / np.sqrt(deg.astype(np.float64))).astype(np.float32)

    # Load-balanced node->block assignment (LPT on message count = deg).
    def lpt_assign(key1, key2=None):
        """Greedy assignment minimizing per-block max of key1 (and key2 as a
        tiebreaker-ish combined potential). Returns node_row."""
        if key2 is None:
            key2 = np.zeros_like(key1)
        order_ = np.argsort(-(key1 + key2), kind="stable")
        rows = np.empty(N, dtype=np.int64)
        heap_ = [(0.0, b) for b in range(nblocks)]
        heapq.heapify(heap_)
        f1 = np.zeros(nblocks, dtype=np.int64)
        f2 = np.zeros(nblocks, dtype=np.int64)
        cnt_ = np.zeros(nblocks, dtype=np.int64)
        # scale so both coordinates contribute comparably to the potential
        s1 = 1.0 / max(1.0, key1.sum() / nblocks)
        s2 = 1.0 / max(1.0, key2.sum() / nblocks) if key2.any() else 0.0
        for n in order_:
            while True:
                f, b = heapq.heappop(heap_)
                if cnt_[b] < 128:
                    break
            rows[n] = b * 128 + cnt_[b]
            cnt_[b] += 1
            f1[b] += key1[n]
            f2[b] += key2[n]
            heapq.heappush(heap_, (f1[b] * s1 + f2[b] * s2, b))
        return rows

    node_row = lpt_assign(deg)
    for _ in range(cfg.balance_iters):
        # lo/hi-aware refinement: classify each edge by its src's current
        # table half, re-balance blocks on (lo_in, hi_in) jointly.
        cc0 = node_row // nlocp
        jj0 = node_row % nlocp
        k0 = cfg.ag_splits
        b0 = [round(i * nb / k0) * 128 for i in range(k0 + 1)]
        tr = np.empty_like(node_row)
        off0 = 0
        for s in range(k0):
            m = (jj0 >= b0[s]) & (jj0 < b0[s + 1])
            sz = b0[s + 1] - b0[s]
            tr[m] = off0 + cc0[m] * sz + (jj0[m] - b0[s])
            off0 += C * sz
        e_lo = tr[src] < cfg.half
        lo_in = np.bincount(dst[e_lo], minlength=N)
        hi_in = np.bincount(dst[~e_lo], minlength=N)
        node_row = lpt_assign(lo_in, hi_in)

    # Table-row numbering for gather indices: with ag_splits=k the AllGather
    # runs as k block-aligned sub-collectives over shard slices, so the
    # physical table is slice-major: slice s holds every core's rows
    # [bnds[s], bnds[s+1]) back-to-back.
    k = cfg.ag_splits
    bnds = [round(i * nb / k) * 128 for i in range(k + 1)]
    cc = node_row // nlocp
    jj = node_row % nlocp
    tabrow = np.empty_like(node_row)
    off = 0
    for s in range(k):
        lo_b, hi_b = bnds[s], bnds[s + 1]
        sz = hi_b - lo_b
        m = (jj >= lo_b) & (jj < hi_b)
        tabrow[m] = off + cc[m] * sz + (jj[m] - lo_b)
        off += C * sz

    # Per-block edge lists (excluding self loops; those are the SBUF chunk).
    srow = tabrow[src]
    drow = node_row[dst]
    eblk = drow // 128
    eslot = drow % 128
    o = np.lexsort((srow, eblk))
    srow_s, eblk_s, eslot_s = srow[o], eblk[o], eslot[o]
    starts = np.searchsorted(eblk_s, np.arange(nblocks + 1))

    lo_counts = np.empty(nblocks, dtype=np.int64)
    hi_counts = np.empty(nblocks, dtype=np.int64)
    for b in range(nblocks):
        s, e = starts[b], starts[b + 1]
        p = np.searchsorted(srow_s[s:e], cfg.half)
        lo_counts[b] = p
        hi_counts[b] = e - s - p
    nch_lo = int(max(1, math.ceil(lo_counts.max() / 128)))
    nch_hi = int(math.ceil(hi_counts.max() / 128)) if ntab > cfg.half else 0
    if ntab > cfg.half:
        nch_hi = max(1, nch_hi)
    nch_e = nch_lo + nch_hi

    # Per-core packed idx (int16, 16-wrap replicated x8) and segid arrays.
    t16 = nb * nch_e * 8  # int16 columns per core
    nsegc = nb * nch_e
    idx_all = np.zeros((C, 128, t16), dtype=np.int16)
    seg_all = np.full((C, 128, nsegc), -1.0, dtype=BF16)

    pad_val = -1 if cfg.neg_pad else 0

    def pack_idx(vals, n_slots):
        a = np.full(n_slots, pad_val, dtype=np.int16)
        a[: len(vals)] = vals
        return a.reshape(n_slots // 16, 16).T  # [16, n16]

    for c in range(C):
        for j in range(nb):
            b = c * nb + j
            s, e = starts[b], starts[b + 1]
            p = lo_counts[b]
            lo_rows = srow_s[s : s + p]
            hi_rows = srow_s[s + p : e] - cfg.half
            lo_seg = eslot_s[s : s + p].astype(np.float32)
            hi_seg = eslot_s[s + p : e].astype(np.float32)
            col0 = j * nch_e * 8
            w16 = pack_idx(lo_rows.astype(np.int16), nch_lo * 128)
            idx_all[c, :, col0 : col0 + nch_lo * 8] = np.tile(w16, (8, 1))
            if nch_hi:
                w16 = pack_idx(hi_rows.astype(np.int16), nch_hi * 128)
                idx_all[c, :, col0 + nch_lo * 8 : col0 + nch_e * 8] = np.tile(
                    w16, (8, 1)
                )
            segc0 = j * nch_e
            for k, segs in ((0, lo_seg), (nch_lo, hi_seg)):
                for ch in range(math.ceil(len(segs) / 128)):
                    chunk = segs[ch * 128 : (ch + 1) * 128]
                    seg_all[c, : len(chunk), segc0 + k + ch] = chunk.astype(BF16)

    # dis by table row (pads -> 0).
    dis_row = np.zeros(ntab, dtype=np.float32)
    dis_row[node_row] = dis
    return dict(
        nloc=nloc,
        nb=nb,
        nlocp=nlocp,
        ntab=ntab,
        nch_lo=nch_lo,
        nch_hi=nch_hi,
        nch_e=nch_e,
        t16=t16,
        nsegc=nsegc,
        node_row=node_row,
        ag_bnds=bnds,
        dis_row=dis_row,
        idx_all=idx_all,
        seg_all=seg_all,
    )


def _build(cfg: GCNConfig, plan):
    """Build the SPMD Bass program (identical across cores; per-core data
    arrives via ExternalInputs)."""
    import concourse.bacc as bacc
    import concourse.tile as tile
    from concourse import mybir

    dt = mybir.dt
    F, L, C = cfg.feat, cfg.n_layers, cfg.n_cores
    nb, nlocp, ntab = plan["nb"], plan["nlocp"], plan["ntab"]
    nloc = plan["nloc"]
    nch_lo, nch_hi, nch_e = plan["nch_lo"], plan["nch_hi"], plan["nch_e"]
    t16, nsegc = plan["t16"], plan["nsegc"]
    half = cfg.half

    nc = bacc.Bacc(
        "TRN2",
        target_bir_lowering=False,
        debug=False,
        num_devices=C,
        num_swdge_queues=cfg.n_swdge_queues,
    )
    RG = [list(range(C))]

    xT_d = nc.dram_tensor("xT", [F, nlocp], dt.bfloat16, kind="ExternalInput")
    W_d = nc.dram_tensor("Wb", [L, F, F], dt.bfloat16, kind="ExternalInput")
    idx_d = nc.dram_tensor("idx", [128, t16], dt.int16, kind="ExternalInput")
    seg_d = nc.dram_tensor("seg", [128, nsegc], dt.bfloat16, kind="ExternalInput")
    disrep_d = nc.dram_tensor("disrep", [128, nlocp], dt.float32, kind="ExternalInput")
    discol_d = nc.dram_tensor("discol", [128, nb], dt.float32, kind="ExternalInput")
    bcol_d = nc.dram_tensor("bcol", [128, L], dt.float32, kind="ExternalInput")
    iota_d = nc.dram_tensor("iota", [128, 128], dt.bfloat16, kind="ExternalInput")
    ident_d = nc.dram_tensor("ident", [128, 128], dt.bfloat16, kind="ExternalInput")
    linw_d = nc.dram_tensor("linw", [F, 1], dt.bfloat16, kind="ExternalInput")
    linb_d = nc.dram_tensor("linb", [1, 1], dt.float32, kind="ExternalInput")
    out_d = nc.dram_tensor("out", [1, nlocp], dt.float32, kind="ExternalOutput")

    bounces = [nc.dram_tensor(f"bounce{i}", [nlocp, F], dt.bfloat16) for i in range(2)]
    tables = [
        nc.dram_tensor(f"table{i}", [ntab, F], dt.bfloat16, addr_space="Shared")
        for i in range(2)
    ]

    with tile.TileContext(nc) as tc:
        with (
            tc.tile_pool(name="const", bufs=1) as const,
            tc.tile_pool(name="gpool", bufs=3) as gpool,
            tc.tile_pool(name="spool", bufs=3) as spool,
            tc.tile_pool(name="tpool", bufs=4) as tpool,
            tc.tile_pool(name="psA", bufs=3, space="PSUM") as psA,
            tc.tile_pool(name="psO", bufs=4, space="PSUM") as psO,
            tc.tile_pool(name="psL", bufs=1, space="PSUM") as psL,
        ):
            # ---- persistent tiles + one-time loads
            h0 = const.tile([F, nlocp], dt.bfloat16, tag="h0")
            h1 = const.tile([F, nlocp], dt.bfloat16, tag="h1")
            zbufs = [
                const.tile([128, nb * F], dt.bfloat16, tag=f"zbuf{i}", name=f"zbuf{i}")
                for i in range(2)
            ]
            W_sb = const.tile([F, L * F], dt.bfloat16, tag="W")
            idx_sb = const.tile([128, t16], dt.int16, tag="idx")
            seg_sb = const.tile([128, nsegc], dt.bfloat16, tag="seg")
            disrep = const.tile([128, nlocp], dt.float32, tag="disrep")
            discol = const.tile([128, nb], dt.float32, tag="discol")
            bcol = const.tile([128, L], dt.float32, tag="bcol")
            iota = const.tile([128, 128], dt.bfloat16, tag="iota")
            ident = const.tile([128, 128], dt.bfloat16, tag="ident")
            linw = const.tile([F, 1], dt.bfloat16, tag="linw")
            linb = const.tile([1, 1], dt.float32, tag="linb")
            orow = const.tile([1, nlocp], dt.float32, tag="orow")

            nc.sync.dma_start(out=h0[:], in_=xT_d[:])
            nc.sync.dma_start(
                out=W_sb[:].rearrange("p (l f) -> p l f", f=F),
                in_=W_d[:].rearrange("l p f -> p l f"),
            )
            nc.sync.dma_start(out=idx_sb[:], in_=idx_d[:])
            nc.sync.dma_start(out=seg_sb[:], in_=seg_d[:])
            nc.sync.dma_start(out=disrep[:], in_=disrep_d[:])
            nc.sync.dma_start(out=discol[:], in_=discol_d[:])
            nc.sync.dma_start(out=bcol[:], in_=bcol_d[:])
            nc.sync.dma_start(out=iota[:], in_=iota_d[:])
            nc.sync.dma_start(out=ident[:], in_=ident_d[:])
            nc.sync.dma_start(out=linw[:], in_=linw_d[:])
            nc.sync.dma_start(out=linb[:], in_=linb_d[:])

            if cfg.neg_pad:
                # -1 pads are skipped by the gather; zero the pool slots once
                # so untouched pad slots hold 0 (not NaN bits) for the matmul.
                for r in range(3):
                    gz = gpool.tile([128, nch_e, F], dt.bfloat16, tag="g",
                                    name=f"gz{r}")
                    nc.vector.memset(gz[:], 0.0)

            hs = [h0, h1]
            swdge_k = [0]  # global Pool-DMA instruction counter: queue must be
            # a pure function of (k mod NUM_SWDGE_GLOBAL_SEMS) so each DMASW
            # sem lane stays locked to one SWDGE queue.
            for l in range(L):
                h_in = hs[l % 2]
                h_out = hs[(l + 1) % 2]
                zbuf = zbufs[l % 2]
                bounce = bounces[l % 2]
                table = tables[l % 2]
                # ---- stage A: Z'' = dis * (h @ W), node-major into zbuf
                for j in range(nb):
                    jsl = slice(j * 128, (j + 1) * 128)
                    zp = psA.tile([128, F], dt.float32, tag="zp")
                    nc.tensor.matmul(
                        out=zp[:],
                        lhsT=h_in[:, jsl],
                        rhs=W_sb[:, l * F : (l + 1) * F],
                        start=True,
                        stop=True,
                    )
                    nc.vector.tensor_scalar_mul(
                        out=zbuf[:, jsl], in0=zp[:], scalar1=discol[:, j : j + 1]
                    )
                bnds = plan["ag_bnds"]
                toff = 0
                for s in range(cfg.ag_splits):
                    lo_b, hi_b = bnds[s], bnds[s + 1]
                    sz = hi_b - lo_b
                    nc.sync.dma_start(
                        out=bounce[lo_b:hi_b, :].rearrange("(b p) f -> p b f", p=128),
                        in_=zbuf[:, lo_b * F // 128 : hi_b * F // 128].rearrange(
                            "p (b f) -> p b f", f=F
                        ),
                    )
                    nc.gpsimd.collective_compute(
                        "AllGather",
                        mybir.AluOpType.bypass,
                        replica_groups=RG,
                        ins=[bounce[lo_b:hi_b, :]],
                        outs=[table[toff : toff + C * sz, :]],
                    )
                    toff += C * sz
                # ---- stage B: per dst-block segment sum
                for j in range(nb):
                    jsl = slice(j * 128, (j + 1) * 128)
                    g = gpool.tile([128, nch_e, F], dt.bfloat16, tag="g")
                    o16 = j * nch_e * 8
                    sub = 0
                    for c0, ncc, tab in (
                        (0, nch_lo, table[0:half, :] if nch_hi else table[:, :]),
                        (nch_lo, nch_hi, table[half:ntab, :]),
                    ):
                        for cs in range(c0, c0 + ncc, cfg.max_gather_chunks):
                            w = min(cfg.max_gather_chunks, c0 + ncc - cs)
                            nc.gpsimd.dma_gather(
                                g[:, cs : cs + w, :],
                                tab,
                                idx_sb[:, o16 + cs * 8 : o16 + (cs + w) * 8],
                                w * 128,
                                w * 128,
                                F,
                                elem_step=F,
                                single_packet=False,
                                queue_num=swdge_k[0] % cfg.n_swdge_queues,
                            )
                            swdge_k[0] += 1
                            sub += 1
                    S = spool.tile([128, nch_e * 128], dt.bfloat16, tag="S")
                    nc.vector.tensor_tensor(
                        out=S[:].rearrange("p (c f) -> p c f", f=128),
                        in0=iota[:].unsqueeze(1).to_broadcast([128, nch_e, 128]),
                        in1=seg_sb[:, j * nch_e : (j + 1) * nch_e]
                        .unsqueeze(2)
                        .to_broadcast([128, nch_e, 128]),
                        op=mybir.AluOpType.is_equal,
                    )
                    O = psO.tile([F, 128], dt.float32, tag="O")
                    nc.tensor.matmul(
                        out=O[:], lhsT=zbuf[:, jsl], rhs=ident[:], start=True,
                        stop=False,
                    )
                    for ch in range(nch_e):
                        nc.tensor.matmul(
                            out=O[:],
                            lhsT=g[:, ch, :],
                            rhs=S[:, ch * 128 : (ch + 1) * 128],
                            start=False,
                            stop=(ch == nch_e - 1),
                        )
                    T = tpool.tile([F, 128], dt.float32, tag="T")
                    nc.vector.tensor_tensor(
                        out=T[:], in0=O[:], in1=disrep[:, jsl],
                        op=mybir.AluOpType.mult,
                    )
                    nc.scalar.activation(
                        out=h_out[:, jsl],
                        in_=T[:],
                        func=mybir.ActivationFunctionType.Relu,
                        bias=bcol[:, l : l + 1],
                        scale=1.0,
                    )
            # ---- final linear readout
            h_fin = hs[L % 2]
            for k in range(0, nlocp, 512):
                w = min(512, nlocp - k)
                op = psL.tile([1, 512], dt.float32, tag="op")
                nc.tensor.matmul(
                    out=op[:, :w], lhsT=linw[:], rhs=h_fin[:, k : k + w],
                    start=True, stop=True,
                )
                nc.scalar.activation(
                    out=orow[:, k : k + w],
                    in_=op[:, :w],
                    func=mybir.ActivationFunctionType.Identity,
                    bias=linb[:],
                    scale=1.0,
                )
            nc.sync.dma_start(out=out_d[:], in_=orow[:])
    nc.compile()
    return nc


def _make_inputs(cfg: GCNConfig, plan, x, Ws, bs, lin_w, lin_b):
    """Per-core in_maps from full inputs + plan."""
    C, F, L = cfg.n_cores, cfg.feat, cfg.n_layers
    nlocp, nb = plan["nlocp"], plan["nb"]
    node_row = plan["node_row"]
    dis_row = plan["dis_row"]

    x = np.asarray(x, dtype=np.float32)
    Ws = np.asarray(Ws, dtype=np.float32)
    bs = np.asarray(bs, dtype=np.float32)
    lin_w = np.asarray(lin_w, dtype=np.float32)
    lin_b = np.asarray(lin_b, dtype=np.float32)

    xrow = np.zeros((C * nlocp, F), dtype=np.float32)
    xrow[node_row] = x
    Wb = Ws.astype(BF16)
    bcol = bs.T.astype(np.float32).copy()  # [F, L]
    iota = np.tile(np.arange(128, dtype=np.float32), (128, 1)).astype(BF16)
    ident = np.eye(128, dtype=np.float32).astype(BF16)
    linw = lin_w.reshape(F, 1).astype(BF16)
    linb = lin_b.reshape(1, 1).astype(np.float32)

    in_maps = []
    for c in range(C):
        rows = slice(c * nlocp, (c + 1) * nlocp)
        dloc = dis_row[rows]
        in_maps.append(
            {
                "xT": np.ascontiguousarray(xrow[rows].T).astype(BF16),
                "Wb": Wb,
                "idx": np.ascontiguousarray(plan["idx_all"][c]),
                "seg": np.ascontiguousarray(plan["seg_all"][c]),
                "disrep": np.tile(dloc, (128, 1)),
                "discol": np.ascontiguousarray(dloc.reshape(nb, 128).T),
                "bcol": bcol,
                "iota": iota,
                "ident": ident,
                "linw": linw,
                "linb": linb,
            }
        )
    return in_maps


def _reassemble(cfg: GCNConfig, plan, outs):
    nlocp = plan["nlocp"]
    node_row = plan["node_row"]
    full = np.zeros(cfg.n_cores * nlocp, dtype=np.float32)
    for c, o in enumerate(outs):
        full[c * nlocp : (c + 1) * nlocp] = o["out"].reshape(-1)
    return full[node_row]


# ---------------------------------------------------------------------------
# v2: variable per-block chunks, batched gather calls, 4 SWDGE queues,
# lo/hi-split AllGather overlap, zero-bias dis-folding.
# ---------------------------------------------------------------------------


def _plan_v2(cfg: GCNConfig, edge_index: np.ndarray):
    import heapq

    N, C = cfg.n_nodes, cfg.n_cores
    nloc = N // C
    nb = (nloc + 127) // 128
    nlocp = nb * 128
    nblocks = C * nb
    B_LO = _env("GCN_BLO", 32)  # blocks per core in the lo table slice
    n_lo = B_LO * 128  # 4096 rows/core -> 32768 total == int16 half
    n_hi = nlocp - n_lo
    # Max chunks per dma_gather call. Keep <=16: 16 chunks = 2048 descriptors
    # = 128/engine, the SWDGE ring capacity; larger calls stall the Q7 in
    # await_space mid-call.
    CALLCH = _env("GCN_CALLCH", 14)

    src = np.asarray(edge_index[0], dtype=np.int64)
    dst = np.asarray(edge_index[1], dtype=np.int64)
    deg = np.bincount(dst, minlength=N).astype(np.int64) + 1
    dis = (1.0 / np.sqrt(deg.astype(np.float64))).astype(np.float32)

    # LPT node->block assignment on in-degree (message count).
    order_ = np.argsort(-deg, kind="stable")
    rows = np.empty(N, dtype=np.int64)
    heap_ = [(0, b) for b in range(nblocks)]
    heapq.heapify(heap_)
    fill = np.zeros(nblocks, dtype=np.int64)
    cnt_ = np.zeros(nblocks, dtype=np.int64)
    for n in order_:
        while True:
            f, b = heapq.heappop(heap_)
            if cnt_[b] < 128:
                break
        rows[n] = b * 128 + cnt_[b]
        cnt_[b] += 1
        fill[b] += deg[n]
        heapq.heappush(heap_, (fill[b], b))
    node_row = rows

    # Lo slice is gathered in two half-slice collectives; number lo table rows
    # sub-slice-major so each sub-AllGather writes a contiguous region.
    n_half = n_lo // 2

    def classify(node_row):
        cc = node_row // nlocp
        jj = node_row % nlocp
        lo_node = jj < n_lo
        sub = jj // n_half  # 0 or 1 within lo (garbage for hi, masked out)
        lo_row = np.minimum(sub, 1) * C * n_half + cc * n_half + (jj % n_half)
        tabrow = np.where(lo_node, lo_row, C * n_lo + cc * n_hi + (jj - n_lo))
        drow = node_row[dst]
        gb = drow // 128
        e_lo = lo_node[src]
        lo_cnt = np.bincount(gb[e_lo], minlength=nblocks)
        hi_cnt = np.bincount(gb[~e_lo], minlength=nblocks)
        return tabrow, lo_cnt, hi_cnt

    # Consumption order. hi-first looked good on paper (hi AllGather issues
    # after ~35% of a layer) but saturates the DMA rings by overlapping
    # gather transfers with the big lo AllGather — measurably slower.
    if _env("GCN_HIFIRST", 0):
        cons_order = list(range(B_LO, nb)) + list(range(B_LO))
    else:
        cons_order = list(range(nb))

    # Relabel blocks within each core by (lo,hi) chunk profile so every core's
    # rank-r block has a near-identical chunk count (one SPMD program), with
    # the lightest blocks first in CONSUMPTION order so the tensor engine can
    # start a layer after minimal gather data.
    for _ in range(2):
        _, lo_cnt, hi_cnt = classify(node_row)
        key = (np.ceil(lo_cnt / 128) * 1000 + np.ceil(hi_cnt / 128)) * 10000 + lo_cnt
        perm = np.empty(nblocks, dtype=np.int64)
        for c in range(C):
            o = np.argsort(key[c * nb : (c + 1) * nb], kind="stable")
            # k-th lightest old block -> consumption position k
            newblk = np.empty(nb, dtype=np.int64)
            newblk[o] = np.asarray(cons_order)
            perm[c * nb : (c + 1) * nb] = c * nb + newblk
        oldblk = node_row // 128
        node_row = perm[oldblk] * 128 + node_row % 128

    tabrow, lo_cnt, hi_cnt = classify(node_row)
    lo_cnt2 = lo_cnt.reshape(C, nb)
    hi_cnt2 = hi_cnt.reshape(C, nb)
    lo_ch = np.ceil(lo_cnt2.max(0) / 128).astype(np.int64)  # shared profile
    hi_ch = np.ceil(hi_cnt2.max(0) / 128).astype(np.int64)
    lo_ch = np.maximum(lo_ch, 1)
    hi_ch = np.maximum(hi_ch, 1)

    # Call packing (shared): ranks in CONSUMPTION order, <= CALLCH chunks per
    # call. offs[r] = absolute chunk offset of rank r in the stream.
    def pack(ch):
        calls = []  # (ranks, chunk_off, nch)
        offs = np.zeros(nb, dtype=np.int64)
        cur, acc, call_off = [], 0, 0
        for r in cons_order:
            if acc + ch[r] > CALLCH and cur:
                calls.append((cur, call_off, acc))
                call_off += acc
                cur, acc = [], 0
            offs[r] = call_off + acc
            cur.append(r)
            acc += ch[r]
        calls.append((cur, call_off, acc))
        return calls, offs, call_off + acc

    lo_calls, lo_offs, tot_lo = pack(lo_ch)
    hi_calls, hi_offs, tot_hi = pack(hi_ch)
    tot_lo, tot_hi = int(tot_lo), int(tot_hi)

    # Per-core idx (int16, 16-wrap x8) and seg streams.
    idx_lo = np.zeros((C, 128, tot_lo * 8), dtype=np.int16)
    idx_hi = np.zeros((C, 128, tot_hi * 8), dtype=np.int16)
    seg_lo = np.full((C, 128, tot_lo), -1.0, dtype=BF16)
    seg_hi = np.full((C, 128, tot_hi), -1.0, dtype=BF16)

    srow_all = tabrow[src]
    drow_all = node_row[dst]
    gb_all = drow_all // 128
    slot_all = drow_all % 128
    e_lo_all = srow_all < C * n_lo
    o = np.lexsort((srow_all, gb_all))
    srow_s, gb_s, slot_s, elo_s = srow_all[o], gb_all[o], slot_all[o], e_lo_all[o]
    starts = np.searchsorted(gb_s, np.arange(nblocks + 1))

    def fill_stream(idx_a, seg_a, c, rr, rows_rel, slots, off_ch, nch):
        ns = nch * 128
        a = np.zeros(ns, dtype=np.int16)
        a[: len(rows_rel)] = rows_rel
        w = a.reshape(nch, 8, 16)
        for k in range(nch):
            idx_a[c, :, (off_ch + k) * 8 : (off_ch + k + 1) * 8] = np.tile(
                w[k].T, (8, 1)
            )
        s = np.full(ns, -1.0, dtype=np.float32)
        s[: len(slots)] = slots
        seg_a[c, :, off_ch : off_ch + nch] = (
            s.reshape(nch, 128).T.astype(BF16)
        )

    for c in range(C):
        for r in range(nb):
            b = c * nb + r
            s0, s1 = starts[b], starts[b + 1]
            m = elo_s[s0:s1]
            p = int(m.sum())
            lo_rows = srow_s[s0:s1][m].astype(np.int16)  # global rows [0, 32768)
            hi_rows = (srow_s[s0:s1][~m] - C * n_lo).astype(np.int16)
            assert p <= lo_ch[r] * 128 and (s1 - s0 - p) <= hi_ch[r] * 128
            fill_stream(idx_lo, seg_lo, c, r, lo_rows, slot_s[s0:s1][m],
                        int(lo_offs[r]), int(lo_ch[r]))
            fill_stream(idx_hi, seg_hi, c, r, hi_rows, slot_s[s0:s1][~m],
                        int(hi_offs[r]), int(hi_ch[r]))

    dis_row = np.zeros(C * nlocp, dtype=np.float32)
    dis_row[node_row] = dis

    # Precompute the one-hot selector matrices host-side (indices are static
    # across layers): S[p, c*128 + f] = (seg[p, c] == f). Streamed from DRAM
    # each layer instead of rebuilding on DVE. fp8e4 represents 0/1 exactly
    # and halves the stream bytes.
    f_iota = np.arange(128, dtype=np.float32)
    s_dtype = ml_dtypes.float8_e4m3fn if _env("GCN_SFP8", 1) else BF16

    def build_S(seg):
        s = np.asarray(seg, dtype=np.float32)  # [C, 128, tot]
        return (s[:, :, :, None] == f_iota).reshape(C, 128, -1).astype(s_dtype)

    S_lo = build_S(seg_lo)
    S_hi = build_S(seg_hi)
    return dict(
        nloc=nloc, nb=nb, nlocp=nlocp, B_LO=B_LO, n_lo=n_lo, n_hi=n_hi,
        lo_ch=lo_ch, hi_ch=hi_ch, lo_offs=lo_offs, hi_offs=hi_offs,
        lo_calls=lo_calls, hi_calls=hi_calls, tot_lo=tot_lo, tot_hi=tot_hi,
        cons_order=cons_order, node_row=node_row, dis_row=dis_row,
        idx_lo=idx_lo, idx_hi=idx_hi, seg_lo=seg_lo, seg_hi=seg_hi,
        S_lo=S_lo, S_hi=S_hi,
    )


def _build_v2(cfg: GCNConfig, plan):
    import concourse.bacc as bacc
    import concourse.tile as tile
    from concourse import mybir

    dt = mybir.dt
    F, L, C = cfg.feat, cfg.n_layers, cfg.n_cores
    nb, nlocp = plan["nb"], plan["nlocp"]
    n_lo, n_hi, B_LO = plan["n_lo"], plan["n_hi"], plan["B_LO"]
    lo_ch, hi_ch = plan["lo_ch"], plan["hi_ch"]
    lo_offs, hi_offs = plan["lo_offs"], plan["hi_offs"]
    lo_calls, hi_calls = plan["lo_calls"], plan["hi_calls"]
    tot_lo, tot_hi = plan["tot_lo"], plan["tot_hi"]
    NQ = cfg.n_swdge_queues

    nc = bacc.Bacc(
        "TRN2",
        target_bir_lowering=False,
        debug=False,
        num_devices=C,
        num_swdge_queues=NQ,
    )
    RG = [list(range(C))]

    xT_d = nc.dram_tensor("xT", [F, nlocp], dt.bfloat16, kind="ExternalInput")
    W_d = nc.dram_tensor("Wb", [L, F, F], dt.bfloat16, kind="ExternalInput")
    idxlo_d = nc.dram_tensor("idxlo", [128, tot_lo * 8], dt.int16, kind="ExternalInput")
    idxhi_d = nc.dram_tensor("idxhi", [128, tot_hi * 8], dt.int16, kind="ExternalInput")
    s_dt = dt.float8e4 if _env("GCN_SFP8", 1) else dt.bfloat16
    Slo_d = nc.dram_tensor("Slo", [128, tot_lo * 128], s_dt, kind="ExternalInput")
    Shi_d = nc.dram_tensor("Shi", [128, tot_hi * 128], s_dt, kind="ExternalInput")
    discol_d = nc.dram_tensor("discol", [128, nb], dt.float32, kind="ExternalInput")
    discol2_d = nc.dram_tensor("discol2", [128, nb], dt.float32, kind="ExternalInput")
    disrow_d = nc.dram_tensor("disrow", [1, nlocp], dt.float32, kind="ExternalInput")
    iota_d = nc.dram_tensor("iota", [128, 128], dt.bfloat16, kind="ExternalInput")
    ident_d = nc.dram_tensor("ident", [128, 128], dt.bfloat16, kind="ExternalInput")
    linw_d = nc.dram_tensor("linw", [F, 1], dt.bfloat16, kind="ExternalInput")
    out_d = nc.dram_tensor("out", [1, nlocp], dt.float32, kind="ExternalOutput")

    bounces_lo = [nc.dram_tensor(f"bnlo{i}", [n_lo, F], dt.bfloat16) for i in range(2)]
    bounces_hi = [nc.dram_tensor(f"bnhi{i}", [n_hi, F], dt.bfloat16) for i in range(2)]
    tables_lo = [
        nc.dram_tensor(f"tablo{i}", [C * n_lo, F], dt.bfloat16, addr_space="Shared")
        for i in range(2)
    ]
    tables_hi = [
        nc.dram_tensor(f"tabhi{i}", [C * n_hi, F], dt.bfloat16, addr_space="Shared")
        for i in range(2)
    ]

    with tile.TileContext(nc) as tc:
        with (
            tc.tile_pool(name="const", bufs=1) as const,
            tc.tile_pool(name="glo", bufs=4) as glo,
            tc.tile_pool(name="ghi", bufs=4) as ghi,
            tc.tile_pool(name="slo", bufs=4) as slo,
            tc.tile_pool(name="shi", bufs=4) as shi,
            tc.tile_pool(name="psA", bufs=2, space="PSUM") as psA,
            tc.tile_pool(name="psO", bufs=5, space="PSUM") as psO,
            tc.tile_pool(name="psL", bufs=1, space="PSUM") as psL,
        ):
            h0 = const.tile([F, nlocp], dt.bfloat16, tag="h0")
            h1 = const.tile([F, nlocp], dt.bfloat16, tag="h1")
            zbufs = [
                const.tile([128, nb * F], dt.bfloat16, tag=f"zbuf{i}", name=f"zbuf{i}")
                for i in range(2)
            ]
            W_sb = const.tile([F, L * F], dt.bfloat16, tag="W")
            idxlo_sb = const.tile([128, tot_lo * 8], dt.int16, tag="idxlo")
            idxhi_sb = const.tile([128, tot_hi * 8], dt.int16, tag="idxhi")
            discol = const.tile([128, nb], dt.float32, tag="discol")
            discol2 = const.tile([128, nb], dt.float32, tag="discol2")
            disrow = const.tile([1, nlocp], dt.float32, tag="disrow")
            iota = const.tile([128, 128], dt.bfloat16, tag="iota")
            ident = const.tile([128, 128], dt.bfloat16, tag="ident")
            linw = const.tile([F, 1], dt.bfloat16, tag="linw")
            orow = const.tile([1, nlocp], dt.float32, tag="orow")

            nc.sync.dma_start(out=h0[:], in_=xT_d[:])
            nc.sync.dma_start(
                out=W_sb[:].rearrange("p (l f) -> p l f", f=F),
                in_=W_d[:].rearrange("l p f -> p l f"),
            )
            nc.sync.dma_start(out=idxlo_sb[:], in_=idxlo_d[:])
            nc.sync.dma_start(out=idxhi_sb[:], in_=idxhi_d[:])
            nc.sync.dma_start(out=discol[:], in_=discol_d[:])
            nc.sync.dma_start(out=discol2[:], in_=discol2_d[:])
            nc.sync.dma_start(out=disrow[:], in_=disrow_d[:])
            nc.sync.dma_start(out=iota[:], in_=iota_d[:])
            nc.sync.dma_start(out=ident[:], in_=ident_d[:])
            nc.sync.dma_start(out=linw[:], in_=linw_d[:])

            hs = [h0, h1]
            swk = [0]

            def stage_a(l, j, h_in):
                zbuf = zbufs[l % 2]
                jsl = slice(j * 128, (j + 1) * 128)
                zp = psA.tile([128, F], dt.float32, tag="zp")
                nc.tensor.matmul(
                    out=zp[:], lhsT=h_in[:, jsl], rhs=W_sb[:, l * F : (l + 1) * F],
                    start=True, stop=True,
                )
                dc = discol if l == 0 else discol2
                nc.vector.tensor_scalar_mul(
                    out=zbuf[:, jsl], in0=zp[:], scalar1=dc[:, j : j + 1]
                )

            def bounce_ag_lo(l, part=None):
                """AllGather the lo slice. part=(b0, b1) gathers blocks
                [b0, b1); table_lo is sub-slice-major so the output region is
                contiguous: rows [C*b0*128, C*b1*128)."""
                if part is None:
                    # table_lo is sub-slice-major; a full gather is two parts
                    bounce_ag_lo(l, (0, B_LO // 2))
                    bounce_ag_lo(l, (B_LO // 2, B_LO))
                    return
                zbuf = zbufs[l % 2]
                b0, b1 = part
                nc.sync.dma_start(
                    out=bounces_lo[l % 2][b0 * 128 : b1 * 128, :].rearrange(
                        "(b p) f -> p b f", p=128
                    ),
                    in_=zbuf[:, b0 * F : b1 * F].rearrange("p (b f) -> p b f", f=F),
                )
                nc.gpsimd.collective_compute(
                    "AllGather", mybir.AluOpType.bypass, replica_groups=RG,
                    ins=[bounces_lo[l % 2][b0 * 128 : b1 * 128, :]],
                    outs=[tables_lo[l % 2][C * b0 * 128 : C * b1 * 128, :]],
                )

            def bounce_ag_hi(l):
                zbuf = zbufs[l % 2]
                nc.sync.dma_start(
                    out=bounces_hi[l % 2][:].rearrange("(b p) f -> p b f", p=128),
                    in_=zbuf[:, B_LO * F :].rearrange("p (b f) -> p b f", f=F),
                )
                nc.gpsimd.collective_compute(
                    "AllGather", mybir.AluOpType.bypass, replica_groups=RG,
                    ins=[bounces_hi[l % 2][:]], outs=[tables_hi[l % 2][:]],
                )

            cons_order = plan["cons_order"]
            HIF = _env("GCN_HIFIRST", 0)
            SPLIT_LO = _env("GCN_SPLITLO", 1) and not HIF
            half_b = B_LO // 2

            def emit_ags(l, k):
                """Issue AG pieces at the right consumption positions."""
                if HIF:
                    if k == nb - B_LO - 1:
                        bounce_ag_hi(l)
                elif SPLIT_LO:
                    if k == half_b - 1:
                        bounce_ag_lo(l, (0, half_b))
                    elif k == B_LO - 1:
                        bounce_ag_lo(l, (half_b, B_LO))
                elif k == B_LO - 1:
                    bounce_ag_lo(l)

            def emit_late_ag(l):
                if HIF:
                    bounce_ag_lo(l)
                else:
                    bounce_ag_hi(l)

            # layer 0 stage A + AG (early slice matches consumption order)
            for k, j in enumerate(cons_order):
                stage_a(0, j, h0)
                emit_ags(0, k)
            emit_late_ag(0)

            for l in range(L):
                zbuf = zbufs[l % 2]
                h_out = hs[(l + 1) % 2]
                tlo, thi = tables_lo[l % 2], tables_hi[l % 2]

                # --- issue gather calls + S builds, interleaved lo/hi
                g_tiles = {}
                s_tiles = {}

                def emit_call(kind, call):
                    ranks, coff, nchc = call
                    idx_sb = idxlo_sb if kind == 0 else idxhi_sb
                    S_d = Slo_d if kind == 0 else Shi_d
                    tab = tlo if kind == 0 else thi
                    pool = glo if kind == 0 else ghi
                    spool_ = slo if kind == 0 else shi
                    g = pool.tile([128, nchc, F], dt.bfloat16, tag=f"g{kind}")
                    nc.gpsimd.dma_gather(
                        g[:], tab[:],
                        idx_sb[:, coff * 8 : (coff + nchc) * 8],
                        nchc * 128, nchc * 128, F, elem_step=F,
                        single_packet=False,
                        queue_num=swk[0] % NQ,
                    )
                    swk[0] += 1
                    S = spool_.tile([128, nchc * 128], s_dt, tag=f"S{kind}")
                    nc.sync.dma_start(
                        out=S[:], in_=S_d[:, coff * 128 : (coff + nchc) * 128]
                    )
                    for r in ranks:
                        g_tiles[(kind, r)] = (g, S, coff)

                # merged emission: the early-AG kind's calls lead; the other
                # kind starts after DELAY calls so the in-order Pool queue
                # doesn't stall on the late AllGather too early.
                DELAY = _env("GCN_DELAY", 6 if HIF else 1)
                lead, trail = (hi_calls, lo_calls) if HIF else (lo_calls, hi_calls)
                lead_k = 1 if HIF else 0
                merged = []
                a_i, b_i = 0, 0
                while a_i < len(lead) or b_i < len(trail):
                    if a_i < len(lead) and (
                        b_i >= len(trail)
                        or a_i - DELAY < b_i * len(lead) / max(1, len(trail))
                    ):
                        merged.append((lead_k, lead[a_i]))
                        a_i += 1
                    else:
                        merged.append((1 - lead_k, trail[b_i]))
                        b_i += 1
                for kind, call in merged:
                    emit_call(kind, call)

                # --- per-block consumption (hi chunks first)
                for k, j in enumerate(cons_order):
                    jsl = slice(j * 128, (j + 1) * 128)
                    O = psO.tile([F, 128], dt.float32, tag="O")
                    nc.tensor.matmul(
                        out=O[:], lhsT=zbuf[:, jsl], rhs=ident[:], start=True,
                        stop=False,
                    )
                    korder = (
                        ((1, hi_offs, hi_ch), (0, lo_offs, lo_ch))
                        if HIF
                        else ((0, lo_offs, lo_ch), (1, hi_offs, hi_ch))
                    )
                    for kind, offs, chs in korder:
                        g, S, coff = g_tiles[(kind, j)]
                        base = int(offs[j]) - coff
                        nch_j = int(chs[j])
                        last = kind == (0 if HIF else 1)
                        for kk in range(nch_j):
                            nc.tensor.matmul(
                                out=O[:],
                                lhsT=g[:, base + kk, :],
                                rhs=S[:, (base + kk) * 128 : (base + kk + 1) * 128],
                                start=False,
                                stop=(last and kk == nch_j - 1),
                            )
                    nc.scalar.activation(
                        out=h_out[:, jsl], in_=O[:],
                        func=mybir.ActivationFunctionType.Relu,
                        scale=1.0,
                    )
                    if l + 1 < L:
                        stage_a(l + 1, j, h_out)
                        emit_ags(l + 1, k)
                if l + 1 < L:
                    emit_late_ag(l + 1)

            # --- readout: out = dis * (linw . h_L)
            h_fin = hs[L % 2]
            for k in range(0, nlocp, 512):
                w = min(512, nlocp - k)
                op = psL.tile([1, 512], dt.float32, tag="op")
                nc.tensor.matmul(
                    out=op[:, :w], lhsT=linw[:], rhs=h_fin[:, k : k + w],
                    start=True, stop=True,
                )
                nc.vector.tensor_tensor(
                    out=orow[:, k : k + w], in0=op[:, :w],
                    in1=disrow[:, k : k + w], op=mybir.AluOpType.mult,
                )
            nc.sync.dma_start(out=out_d[:], in_=orow[:])
    nc.compile()
    return nc


def _make_inputs_v2(cfg: GCNConfig, plan, x, Ws, bs, lin_w, lin_b):
    C, F, L = cfg.n_cores, cfg.feat, cfg.n_layers
    nlocp, nb = plan["nlocp"], plan["nb"]
    node_row = plan["node_row"]
    dis_row = plan["dis_row"]

    x = np.asarray(x, dtype=np.float32)
    Ws = np.asarray(Ws, dtype=np.float32)

    xrow = np.zeros((C * nlocp, F), dtype=np.float32)
    xrow[node_row] = x
    Wb = Ws.astype(BF16)
    iota = np.tile(np.arange(128, dtype=np.float32), (128, 1)).astype(BF16)
    ident = np.eye(128, dtype=np.float32).astype(BF16)
    linw = np.asarray(lin_w, dtype=np.float32).reshape(F, 1).astype(BF16)

    in_maps = []
    for c in range(C):
        rows = slice(c * nlocp, (c + 1) * nlocp)
        dloc = dis_row[rows]
        dcol = np.ascontiguousarray(dloc.reshape(nb, 128).T)
        in_maps.append(
            {
                "xT": np.ascontiguousarray(xrow[rows].T).astype(BF16),
                "Wb": Wb,
                "idxlo": np.ascontiguousarray(plan["idx_lo"][c]),
                "idxhi": np.ascontiguousarray(plan["idx_hi"][c]),
                "Slo": np.ascontiguousarray(plan["S_lo"][c]),
                "Shi": np.ascontiguousarray(plan["S_hi"][c]),
                "discol": dcol,
                "discol2": dcol * dcol,
                "disrow": dloc.reshape(1, nlocp),
                "iota": iota,
                "ident": ident,
                "linw": linw,
            }
        )
    return in_maps


def kernel(**inputs) -> np.ndarray:
    cfg = GCNConfig()
    return _kernel_impl(cfg, inputs, mode=os.environ.get("GCN_MODE", "hw"))


def _kernel_impl(cfg: GCNConfig, inputs, mode="hw", trace=False):
    x = np.asarray(inputs["x"])
    edge_index = np.asarray(inputs["edge_index"])
    use_v2 = (
        _env("GCN_V2", 1)
        and not np.any(np.asarray(inputs["bs"]))
        and not np.any(np.asarray(inputs["lin_b"]))
    )
    if use_v2:
        cfg.n_swdge_queues = _env("GCN_NSWQ", 4)
        plan = _plan_v2(cfg, edge_index)
        nc = _build_v2(cfg, plan)
        in_maps = _make_inputs_v2(
            cfg, plan, x, inputs["Ws"], inputs["bs"], inputs["lin_w"], inputs["lin_b"]
        )
    else:
        plan = _plan(cfg, edge_index)
        nc = _build(cfg, plan)
        in_maps = _make_inputs(
            cfg, plan, x, inputs["Ws"], inputs["bs"], inputs["lin_w"], inputs["lin_b"]
        )
    if mode == "sim":
        from concourse import bass_interp

        sim = bass_interp.MultiCoreSim(nc, cfg.n_cores)
        for c in range(cfg.n_cores):
            for k, v in in_maps[c].items():
                sim.cores[c].tensor(k)[:] = v
        sim.simulate()
        outs = [
            {"out": np.asarray(sim.cores[c].mem_tensor("out"))}
            for c in range(cfg.n_cores)
        ]
        result = _reassemble(cfg, plan, outs)
        return result.astype(np.float32)
    else:
        from concourse.bass_utils import run_bass_kernel_spmd

        res = run_bass_kernel_spmd(
            nc, in_maps, core_ids=list(range(cfg.n_cores)), trace=trace
        )
        out = _reassemble(cfg, plan, res.results)
        if trace:
            return out.astype(np.float32), res
        return out.astype(np.float32)


if __name__ == "__main__":
    pass



# revision 45
# speedup vs baseline: 1.0425x; 1.0425x over previous
"""GCN (7-layer, PyG-style symmetric-normalized message passing) on 8 TRN2
NeuronCores via Bass/Tile.

Strategy (graph/data parallel over nodes):
  - Nodes are assigned to 8 cores x 49 blocks of 128 slots each (load-balanced
    by in-degree so per-block message counts are nearly equal).
  - Per layer, per core:
      stage A: Z'' = dis * (h @ W) for the core's 6272 node slots (PE matmul
               per 128-node block + per-partition scale), node-major in SBUF.
      AllGather: bf16 Z'' shards -> full 50176-row table in local HBM.
      stage B: per dst block, gather Z''[src] rows for the block's edges
               (SWDGE dma_gather, int16 indices, table split in two halves to
               fit int16 range), build one-hot selector S on DVE
               (S[msg, slot] = (iota == segid)), and segment-sum on PE:
               O[feat, slot] += M_chunk.T @ S_chunk, with the self-loop chunk
               done as zbuf_block.T @ I directly from SBUF.
      epilogue: h' = relu(O * dis[dst] + b)  (DVE mult + ACT relu w/ bias).
  - Final: out = lin_w.T @ h7 + lin_b, one row per core, host reassembles.

All index/normalization prep is host-side numpy (graph routing), baked into
per-core input tensors; the float pipeline runs on device.
"""

import math
import os
import sys
from dataclasses import dataclass

import numpy as np

sys.path.insert(0, "/opt/trn_rl_repo")

import ml_dtypes  # noqa: E402

BF16 = ml_dtypes.bfloat16


def _env(name, default):
    return int(os.environ.get(name, default))


@dataclass
class GCNConfig:
    n_nodes: int = 50000
    n_edges: int = 600000
    feat: int = 128
    n_layers: int = 7
    n_cores: int = 8
    half: int = 32768  # int16 addressable rows per gather table half
    max_gather_chunks: int = _env("GCN_MAXCH", 32)  # max 128-idx chunks per dma_gather call
    n_swdge_queues: int = _env("GCN_NSWQ", 1)  # parallel SWDGE desc-gen queues (1..4)
    neg_pad: bool = False  # pad gather idx lists with -1 (skipped) vs 0
    ag_splits: int = _env("GCN_AGSPL", 1)  # split the per-layer AllGather into this many pieces
    balance_iters: int = _env("GCN_BAL", 0)  # lo/hi-aware node assignment refinement passes


def _plan(cfg: GCNConfig, edge_index: np.ndarray):
    """Host graph prep: node->(core,block,slot) assignment, per-block sorted
    edge lists split by table half, padding, and all static counts."""
    import heapq

    N, C = cfg.n_nodes, cfg.n_cores
    nloc = N // C
    nb = (nloc + 127) // 128
    nlocp = nb * 128
    ntab = nlocp * C
    nblocks = C * nb

    src = np.asarray(edge_index[0], dtype=np.int64)
    dst = np.asarray(edge_index[1], dtype=np.int64)
    deg = np.bincount(dst, minlength=N).astype(np.int64) + 1  # + self loop
    dis = (1.0 / np.sqrt(deg.astype(np.float64))).astype(np.float32)

    # Load-balanced node->block assignment (LPT on message count = deg).
    def lpt_assign(key1, key2=None):
        """Greedy assignment minimizing per-block max of key1 (and key2 as a
        tiebreaker-ish combined potential). Returns node_row."""
        if key2 is None:
            key2 = np.zeros_like(key1)
        order_ = np.argsort(-(key1 + key2), kind="stable")
        rows = np.empty(N, dtype=np.int64)
        heap_ = [(0.0, b) for b in range(nblocks)]
        heapq.heapify(heap_)
        f1 = np.zeros(nblocks, dtype=np.int64)
        f2 = np.zeros(nblocks, dtype=np.int64)
        cnt_ = np.zeros(nblocks, dtype=np.int64)
        # scale so both coordinates contribute comparably to the potential
        s1 = 1.0 / max(1.0, key1.sum() / nblocks)
        s2 = 1.0 / max(1.0, key2.sum() / nblocks) if key2.any() else 0.0
        for n in order_:
            while True:
                f, b = heapq.heappop(heap_)
                if cnt_[b] < 128:
                    break
            rows[n] = b * 128 + cnt_[b]
            cnt_[b] += 1
            f1[b] += key1[n]
            f2[b] += key2[n]
            heapq.heappush(heap_, (f1[b] * s1 + f2[b] * s2, b))
        return rows

    node_row = lpt_assign(deg)
    for _ in range(cfg.balance_iters):
        # lo/hi-aware refinement: classify each edge by its src's current
        # table half, re-balance blocks on (lo_in, hi_in) jointly.
        cc0 = node_row // nlocp
        jj0 = node_row % nlocp
        k0 = cfg.ag_splits
        b0 = [round(i * nb / k0) * 128 for i in range(k0 + 1)]
        tr = np.empty_like(node_row)
        off0 = 0
        for s in range(k0):
            m = (jj0 >= b0[s]) & (jj0 < b0[s + 1])
            sz = b0[s + 1] - b0[s]
            tr[m] = off0 + cc0[m] * sz + (jj0[m] - b0[s])
            off0 += C * sz
        e_lo = tr[src] < cfg.half
        lo_in = np.bincount(dst[e_lo], minlength=N)
        hi_in = np.bincount(dst[~e_lo], minlength=N)
        node_row = lpt_assign(lo_in, hi_in)

    # Table-row numbering for gather indices: with ag_splits=k the AllGather
    # runs as k block-aligned sub-collectives over shard slices, so the
    # physical table is slice-major: slice s holds every core's rows
    # [bnds[s], bnds[s+1]) back-to-back.
    k = cfg.ag_splits
    bnds = [round(i * nb / k) * 128 for i in range(k + 1)]
    cc = node_row // nlocp
    jj = node_row % nlocp
    tabrow = np.empty_like(node_row)
    off = 0
    for s in range(k):
        lo_b, hi_b = bnds[s], bnds[s + 1]
        sz = hi_b - lo_b
        m = (jj >= lo_b) & (jj < hi_b)
        tabrow[m] = off + cc[m] * sz + (jj[m] - lo_b)
        off += C * sz

    # Per-block edge lists (excluding self loops; those are the SBUF chunk).
    srow = tabrow[src]
    drow = node_row[dst]
    eblk = drow // 128
    eslot = drow % 128
    o = np.lexsort((srow, eblk))
    srow_s, eblk_s, eslot_s = srow[o], eblk[o], eslot[o]
    starts = np.searchsorted(eblk_s, np.arange(nblocks + 1))

    lo_counts = np.empty(nblocks, dtype=np.int64)
    hi_counts = np.empty(nblocks, dtype=np.int64)
    for b in range(nblocks):
        s, e = starts[b], starts[b + 1]
        p = np.searchsorted(srow_s[s:e], cfg.half)
        lo_counts[b] = p
        hi_counts[b] = e - s - p
    nch_lo = int(max(1, math.ceil(lo_counts.max() / 128)))
    nch_hi = int(math.ceil(hi_counts.max() / 128)) if ntab > cfg.half else 0
    if ntab > cfg.half:
        nch_hi = max(1, nch_hi)
    nch_e = nch_lo + nch_hi

    # Per-core packed idx (int16, 16-wrap replicated x8) and segid arrays.
    t16 = nb * nch_e * 8  # int16 columns per core
    nsegc = nb * nch_e
    idx_all = np.zeros((C, 128, t16), dtype=np.int16)
    seg_all = np.full((C, 128, nsegc), -1.0, dtype=BF16)

    pad_val = -1 if cfg.neg_pad else 0

    def pack_idx(vals, n_slots):
        a = np.full(n_slots, pad_val, dtype=np.int16)
        a[: len(vals)] = vals
        return a.reshape(n_slots // 16, 16).T  # [16, n16]

    for c in range(C):
        for j in range(nb):
            b = c * nb + j
            s, e = starts[b], starts[b + 1]
            p = lo_counts[b]
            lo_rows = srow_s[s : s + p]
            hi_rows = srow_s[s + p : e] - cfg.half
            lo_seg = eslot_s[s : s + p].astype(np.float32)
            hi_seg = eslot_s[s + p : e].astype(np.float32)
            col0 = j * nch_e * 8
            w16 = pack_idx(lo_rows.astype(np.int16), nch_lo * 128)
            idx_all[c, :, col0 : col0 + nch_lo * 8] = np.tile(w16, (8, 1))
            if nch_hi:
                w16 = pack_idx(hi_rows.astype(np.int16), nch_hi * 128)
                idx_all[c, :, col0 + nch_lo * 8 : col0 + nch_e * 8] = np.tile(
                    w16, (8, 1)
                )
            segc0 = j * nch_e
            for k, segs in ((0, lo_seg), (nch_lo, hi_seg)):
                for ch in range(math.ceil(len(segs) / 128)):
                    chunk = segs[ch * 128 : (ch + 1) * 128]
                    seg_all[c, : len(chunk), segc0 + k + ch] = chunk.astype(BF16)

    # dis by table row (pads -> 0).
    dis_row = np.zeros(ntab, dtype=np.float32)
    dis_row[node_row] = dis
    return dict(
        nloc=nloc,
        nb=nb,
        nlocp=nlocp,
        ntab=ntab,
        nch_lo=nch_lo,
        nch_hi=nch_hi,
        nch_e=nch_e,
        t16=t16,
        nsegc=nsegc,
        node_row=node_row,
        ag_bnds=bnds,
        dis_row=dis_row,
        idx_all=idx_all,
        seg_all=seg_all,
    )


def _build(cfg: GCNConfig, plan):
    """Build the SPMD Bass program (identical across cores; per-core data
    arrives via ExternalInputs)."""
    import concourse.bacc as bacc
    import concourse.tile as tile
    from concourse import mybir

    dt = mybir.dt
    F, L, C = cfg.feat, cfg.n_layers, cfg.n_cores
    nb, nlocp, ntab = plan["nb"], plan["nlocp"], plan["ntab"]
    nloc = plan["nloc"]
    nch_lo, nch_hi, nch_e = plan["nch_lo"], plan["nch_hi"], plan["nch_e"]
    t16, nsegc = plan["t16"], plan["nsegc"]
    half = cfg.half

    nc = bacc.Bacc(
        "TRN2",
        target_bir_lowering=False,
        debug=False,
        num_devices=C,
        num_swdge_queues=cfg.n_swdge_queues,
    )
    RG = [list(range(C))]

    xT_d = nc.dram_tensor("xT", [F, nlocp], dt.bfloat16, kind="ExternalInput")
    W_d = nc.dram_tensor("Wb", [L, F, F], dt.bfloat16, kind="ExternalInput")
    idx_d = nc.dram_tensor("idx", [128, t16], dt.int16, kind="ExternalInput")
    seg_d = nc.dram_tensor("seg", [128, nsegc], dt.bfloat16, kind="ExternalInput")
    disrep_d = nc.dram_tensor("disrep", [128, nlocp], dt.float32, kind="ExternalInput")
    discol_d = nc.dram_tensor("discol", [128, nb], dt.float32, kind="ExternalInput")
    bcol_d = nc.dram_tensor("bcol", [128, L], dt.float32, kind="ExternalInput")
    iota_d = nc.dram_tensor("iota", [128, 128], dt.bfloat16, kind="ExternalInput")
    ident_d = nc.dram_tensor("ident", [128, 128], dt.bfloat16, kind="ExternalInput")
    linw_d = nc.dram_tensor("linw", [F, 1], dt.bfloat16, kind="ExternalInput")
    linb_d = nc.dram_tensor("linb", [1, 1], dt.float32, kind="ExternalInput")
    out_d = nc.dram_tensor("out", [1, nlocp], dt.float32, kind="ExternalOutput")

    bounces = [nc.dram_tensor(f"bounce{i}", [nlocp, F], dt.bfloat16) for i in range(2)]
    tables = [
        nc.dram_tensor(f"table{i}", [ntab, F], dt.bfloat16, addr_space="Shared")
        for i in range(2)
    ]

    with tile.TileContext(nc) as tc:
        with (
            tc.tile_pool(name="const", bufs=1) as const,
            tc.tile_pool(name="gpool", bufs=3) as gpool,
            tc.tile_pool(name="spool", bufs=3) as spool,
            tc.tile_pool(name="tpool", bufs=4) as tpool,
            tc.tile_pool(name="psA", bufs=3, space="PSUM") as psA,
            tc.tile_pool(name="psO", bufs=4, space="PSUM") as psO,
            tc.tile_pool(name="psL", bufs=1, space="PSUM") as psL,
        ):
            # ---- persistent tiles + one-time loads
            h0 = const.tile([F, nlocp], dt.bfloat16, tag="h0")
            h1 = const.tile([F, nlocp], dt.bfloat16, tag="h1")
            zbufs = [
                const.tile([128, nb * F], dt.bfloat16, tag=f"zbuf{i}", name=f"zbuf{i}")
                for i in range(2)
            ]
            W_sb = const.tile([F, L * F], dt.bfloat16, tag="W")
            idx_sb = const.tile([128, t16], dt.int16, tag="idx")
            seg_sb = const.tile([128, nsegc], dt.bfloat16, tag="seg")
            disrep = const.tile([128, nlocp], dt.float32, tag="disrep")
            discol = const.tile([128, nb], dt.float32, tag="discol")
            bcol = const.tile([128, L], dt.float32, tag="bcol")
            iota = const.tile([128, 128], dt.bfloat16, tag="iota")
            ident = const.tile([128, 128], dt.bfloat16, tag="ident")
            linw = const.tile([F, 1], dt.bfloat16, tag="linw")
            linb = const.tile([1, 1], dt.float32, tag="linb")
            orow = const.tile([1, nlocp], dt.float32, tag="orow")

            nc.sync.dma_start(out=h0[:], in_=xT_d[:])
            nc.sync.dma_start(
                out=W_sb[:].rearrange("p (l f) -> p l f", f=F),
                in_=W_d[:].rearrange("l p f -> p l f"),
            )
            nc.sync.dma_start(out=idx_sb[:], in_=idx_d[:])
            nc.sync.dma_start(out=seg_sb[:], in_=seg_d[:])
            nc.sync.dma_start(out=disrep[:], in_=disrep_d[:])
            nc.sync.dma_start(out=discol[:], in_=discol_d[:])
            nc.sync.dma_start(out=bcol[:], in_=bcol_d[:])
            nc.sync.dma_start(out=iota[:], in_=iota_d[:])
            nc.sync.dma_start(out=ident[:], in_=ident_d[:])
            nc.sync.dma_start(out=linw[:], in_=linw_d[:])
            nc.sync.dma_start(out=linb[:], in_=linb_d[:])

            if cfg.neg_pad:
                # -1 pads are skipped by the gather; zero the pool slots once
                # so untouched pad slots hold 0 (not NaN bits) for the matmul.
                for r in range(3):
                    gz = gpool.tile([128, nch_e, F], dt.bfloat16, tag="g",
                                    name=f"gz{r}")
                    nc.vector.memset(gz[:], 0.0)

            hs = [h0, h1]
            swdge_k = [0]  # global Pool-DMA instruction counter: queue must be
            # a pure function of (k mod NUM_SWDGE_GLOBAL_SEMS) so each DMASW
            # sem lane stays locked to one SWDGE queue.
            for l in range(L):
                h_in = hs[l % 2]
                h_out = hs[(l + 1) % 2]
                zbuf = zbufs[l % 2]
                bounce = bounces[l % 2]
                table = tables[l % 2]
                # ---- stage A: Z'' = dis * (h @ W), node-major into zbuf
                for j in range(nb):
                    jsl = slice(j * 128, (j + 1) * 128)
                    zp = psA.tile([128, F], dt.float32, tag="zp")
                    nc.tensor.matmul(
                        out=zp[:],
                        lhsT=h_in[:, jsl],
                        rhs=W_sb[:, l * F : (l + 1) * F],
                        start=True,
                        stop=True,
                    )
                    nc.vector.tensor_scalar_mul(
                        out=zbuf[:, jsl], in0=zp[:], scalar1=discol[:, j : j + 1]
                    )
                bnds = plan["ag_bnds"]
                toff = 0
                for s in range(cfg.ag_splits):
                    lo_b, hi_b = bnds[s], bnds[s + 1]
                    sz = hi_b - lo_b
                    nc.sync.dma_start(
                        out=bounce[lo_b:hi_b, :].rearrange("(b p) f -> p b f", p=128),
                        in_=zbuf[:, lo_b * F // 128 : hi_b * F // 128].rearrange(
                            "p (b f) -> p b f", f=F
                        ),
                    )
                    nc.gpsimd.collective_compute(
                        "AllGather",
                        mybir.AluOpType.bypass,
                        replica_groups=RG,
                        ins=[bounce[lo_b:hi_b, :]],
                        outs=[table[toff : toff + C * sz, :]],
                    )
                    toff += C * sz
                # ---- stage B: per dst-block segment sum
                for j in range(nb):
                    jsl = slice(j * 128, (j + 1) * 128)
                    g = gpool.tile([128, nch_e, F], dt.bfloat16, tag="g")
                    o16 = j * nch_e * 8
                    sub = 0
                    for c0, ncc, tab in (
                        (0, nch_lo, table[0:half, :] if nch_hi else table[:, :]),
                        (nch_lo, nch_hi, table[half:ntab, :]),
                    ):
                        for cs in range(c0, c0 + ncc, cfg.max_gather_chunks):
                            w = min(cfg.max_gather_chunks, c0 + ncc - cs)
                            nc.gpsimd.dma_gather(
                                g[:, cs : cs + w, :],
                                tab,
                                idx_sb[:, o16 + cs * 8 : o16 + (cs + w) * 8],
                                w * 128,
                                w * 128,
                                F,
                                elem_step=F,
                                single_packet=False,
                                queue_num=swdge_k[0] % cfg.n_swdge_queues,
                            )
                            swdge_k[0] += 1
                            sub += 1
                    S = spool.tile([128, nch_e * 128], dt.bfloat16, tag="S")
                    nc.vector.tensor_tensor(
                        out=S[:].rearrange("p (c f) -> p c f", f=128),
                        in0=iota[:].unsqueeze(1).to_broadcast([128, nch_e, 128]),
                        in1=seg_sb[:, j * nch_e : (j + 1) * nch_e]
                        .unsqueeze(2)
                        .to_broadcast([128, nch_e, 128]),
                        op=mybir.AluOpType.is_equal,
                    )
                    O = psO.tile([F, 128], dt.float32, tag="O")
                    nc.tensor.matmul(
                        out=O[:], lhsT=zbuf[:, jsl], rhs=ident[:], start=True,
                        stop=False,
                    )
                    for ch in range(nch_e):
                        nc.tensor.matmul(
                            out=O[:],
                            lhsT=g[:, ch, :],
                            rhs=S[:, ch * 128 : (ch + 1) * 128],
                            start=False,
                            stop=(ch == nch_e - 1),
                        )
                    T = tpool.tile([F, 128], dt.float32, tag="T")
                    nc.vector.tensor_tensor(
                        out=T[:], in0=O[:], in1=disrep[:, jsl],
                        op=mybir.AluOpType.mult,
                    )
                    nc.scalar.activation(
                        out=h_out[:, jsl],
                        in_=T[:],
                        func=mybir.ActivationFunctionType.Relu,
                        bias=bcol[:, l : l + 1],
                        scale=1.0,
                    )
            # ---- final linear readout
            h_fin = hs[L % 2]
            for k in range(0, nlocp, 512):
                w = min(512, nlocp - k)
                op = psL.tile([1, 512], dt.float32, tag="op")
                nc.tensor.matmul(
                    out=op[:, :w], lhsT=linw[:], rhs=h_fin[:, k : k + w],
                    start=True, stop=True,
                )
                nc.scalar.activation(
                    out=orow[:, k : k + w],
                    in_=op[:, :w],
                    func=mybir.ActivationFunctionType.Identity,
                    bias=linb[:],
                    scale=1.0,
                )
            nc.sync.dma_start(out=out_d[:], in_=orow[:])
    nc.compile()
    return nc


def _make_inputs(cfg: GCNConfig, plan, x, Ws, bs, lin_w, lin_b):
    """Per-core in_maps from full inputs + plan."""
    C, F, L = cfg.n_cores, cfg.feat, cfg.n_layers
    nlocp, nb = plan["nlocp"], plan["nb"]
    node_row = plan["node_row"]
    dis_row = plan["dis_row"]

    x = np.asarray(x, dtype=np.float32)
    Ws = np.asarray(Ws, dtype=np.float32)
    bs = np.asarray(bs, dtype=np.float32)
    lin_w = np.asarray(lin_w, dtype=np.float32)
    lin_b = np.asarray(lin_b, dtype=np.float32)

    xrow = np.zeros((C * nlocp, F), dtype=np.float32)
    xrow[node_row] = x
    Wb = Ws.astype(BF16)
    bcol = bs.T.astype(np.float32).copy()  # [F, L]
    iota = np.tile(np.arange(128, dtype=np.float32), (128, 1)).astype(BF16)
    ident = np.eye(128, dtype=np.float32).astype(BF16)
    linw = lin_w.reshape(F, 1).astype(BF16)
    linb = lin_b.reshape(1, 1).astype(np.float32)

    in_maps = []
    for c in range(C):
        rows = slice(c * nlocp, (c + 1) * nlocp)
        dloc = dis_row[rows]
        in_maps.append(
            {
                "xT": np.ascontiguousarray(xrow[rows].T).astype(BF16),
                "Wb": Wb,
                "idx": np.ascontiguousarray(plan["idx_all"][c]),
                "seg": np.ascontiguousarray(plan["seg_all"][c]),
                "disrep": np.tile(dloc, (128, 1)),
                "discol": np.ascontiguousarray(dloc.reshape(nb, 128).T),
                "bcol": bcol,
                "iota": iota,
                "ident": ident,
                "linw": linw,
                "linb": linb,
            }
        )
    return in_maps


def _reassemble(cfg: GCNConfig, plan, outs):
    nlocp = plan["nlocp"]
    node_row = plan["node_row"]
    full = np.zeros(cfg.n_cores * nlocp, dtype=np.float32)
    for c, o in enumerate(outs):
        full[c * nlocp : (c + 1) * nlocp] = o["out"].reshape(-1)
    return full[node_row]


# ---------------------------------------------------------------------------
# v2: variable per-block chunks, batched gather calls, 4 SWDGE queues,
# lo/hi-split AllGather overlap, zero-bias dis-folding.
# ---------------------------------------------------------------------------


def _plan_v2(cfg: GCNConfig, edge_index: np.ndarray):
    import heapq

    N, C = cfg.n_nodes, cfg.n_cores
    nloc = N // C
    nb = (nloc + 127) // 128
    nlocp = nb * 128
    nblocks = C * nb
    B_LO = _env("GCN_BLO", 32)  # blocks per core in the lo table slice
    n_lo = B_LO * 128  # 4096 rows/core -> 32768 total == int16 half
    n_hi = nlocp - n_lo
    # Max chunks per dma_gather call. Keep <=16: 16 chunks = 2048 descriptors
    # = 128/engine, the SWDGE ring capacity; larger calls stall the Q7 in
    # await_space mid-call.
    CALLCH = _env("GCN_CALLCH", 14)

    src = np.asarray(edge_index[0], dtype=np.int64)
    dst = np.asarray(edge_index[1], dtype=np.int64)
    deg = np.bincount(dst, minlength=N).astype(np.int64) + 1
    dis = (1.0 / np.sqrt(deg.astype(np.float64))).astype(np.float32)

    # LPT node->block assignment on in-degree (message count).
    order_ = np.argsort(-deg, kind="stable")
    rows = np.empty(N, dtype=np.int64)
    heap_ = [(0, b) for b in range(nblocks)]
    heapq.heapify(heap_)
    fill = np.zeros(nblocks, dtype=np.int64)
    cnt_ = np.zeros(nblocks, dtype=np.int64)
    for n in order_:
        while True:
            f, b = heapq.heappop(heap_)
            if cnt_[b] < 128:
                break
        rows[n] = b * 128 + cnt_[b]
        cnt_[b] += 1
        fill[b] += deg[n]
        heapq.heappush(heap_, (fill[b], b))
    node_row = rows

    # Lo slice is gathered in two half-slice collectives; number lo table rows
    # sub-slice-major so each sub-AllGather writes a contiguous region.
    n_half = n_lo // 2

    def classify(node_row):
        cc = node_row // nlocp
        jj = node_row % nlocp
        lo_node = jj < n_lo
        sub = jj // n_half  # 0 or 1 within lo (garbage for hi, masked out)
        lo_row = np.minimum(sub, 1) * C * n_half + cc * n_half + (jj % n_half)
        tabrow = np.where(lo_node, lo_row, C * n_lo + cc * n_hi + (jj - n_lo))
        drow = node_row[dst]
        gb = drow // 128
        e_lo = lo_node[src]
        lo_cnt = np.bincount(gb[e_lo], minlength=nblocks)
        hi_cnt = np.bincount(gb[~e_lo], minlength=nblocks)
        return tabrow, lo_cnt, hi_cnt

    # Consumption order. hi-first looked good on paper (hi AllGather issues
    # after ~35% of a layer) but saturates the DMA rings by overlapping
    # gather transfers with the big lo AllGather — measurably slower.
    if _env("GCN_HIFIRST", 0):
        cons_order = list(range(B_LO, nb)) + list(range(B_LO))
    else:
        cons_order = list(range(nb))

    # Relabel blocks within each core by (lo,hi) chunk profile so every core's
    # rank-r block has a near-identical chunk count (one SPMD program), with
    # the lightest blocks first in CONSUMPTION order so the tensor engine can
    # start a layer after minimal gather data.
    for _ in range(2):
        _, lo_cnt, hi_cnt = classify(node_row)
        key = (np.ceil(lo_cnt / 128) * 1000 + np.ceil(hi_cnt / 128)) * 10000 + lo_cnt
        perm = np.empty(nblocks, dtype=np.int64)
        for c in range(C):
            o = np.argsort(key[c * nb : (c + 1) * nb], kind="stable")
            # k-th lightest old block -> consumption position k
            newblk = np.empty(nb, dtype=np.int64)
            newblk[o] = np.asarray(cons_order)
            perm[c * nb : (c + 1) * nb] = c * nb + newblk
        oldblk = node_row // 128
        node_row = perm[oldblk] * 128 + node_row % 128

    tabrow, lo_cnt, hi_cnt = classify(node_row)
    lo_cnt2 = lo_cnt.reshape(C, nb)
    hi_cnt2 = hi_cnt.reshape(C, nb)
    lo_ch = np.ceil(lo_cnt2.max(0) / 128).astype(np.int64)  # shared profile
    hi_ch = np.ceil(hi_cnt2.max(0) / 128).astype(np.int64)
    lo_ch = np.maximum(lo_ch, 1)
    hi_ch = np.maximum(hi_ch, 1)

    # Call packing (shared): ranks in CONSUMPTION order, <= CALLCH chunks per
    # call. offs[r] = absolute chunk offset of rank r in the stream.
    def pack(ch):
        calls = []  # (ranks, chunk_off, nch)
        offs = np.zeros(nb, dtype=np.int64)
        cur, acc, call_off = [], 0, 0
        for r in cons_order:
            if acc + ch[r] > CALLCH and cur:
                calls.append((cur, call_off, acc))
                call_off += acc
                cur, acc = [], 0
            offs[r] = call_off + acc
            cur.append(r)
            acc += ch[r]
        calls.append((cur, call_off, acc))
        return calls, offs, call_off + acc

    lo_calls, lo_offs, tot_lo = pack(lo_ch)
    hi_calls, hi_offs, tot_hi = pack(hi_ch)
    tot_lo, tot_hi = int(tot_lo), int(tot_hi)

    # Per-core idx (int16, 16-wrap x8) and seg streams.
    idx_lo = np.zeros((C, 128, tot_lo * 8), dtype=np.int16)
    idx_hi = np.zeros((C, 128, tot_hi * 8), dtype=np.int16)
    seg_lo = np.full((C, 128, tot_lo), -1.0, dtype=BF16)
    seg_hi = np.full((C, 128, tot_hi), -1.0, dtype=BF16)

    srow_all = tabrow[src]
    drow_all = node_row[dst]
    gb_all = drow_all // 128
    slot_all = drow_all % 128
    e_lo_all = srow_all < C * n_lo
    o = np.lexsort((srow_all, gb_all))
    srow_s, gb_s, slot_s, elo_s = srow_all[o], gb_all[o], slot_all[o], e_lo_all[o]
    starts = np.searchsorted(gb_s, np.arange(nblocks + 1))

    def fill_stream(idx_a, seg_a, c, rr, rows_rel, slots, off_ch, nch):
        ns = nch * 128
        a = np.zeros(ns, dtype=np.int16)
        a[: len(rows_rel)] = rows_rel
        w = a.reshape(nch, 8, 16)
        for k in range(nch):
            idx_a[c, :, (off_ch + k) * 8 : (off_ch + k + 1) * 8] = np.tile(
                w[k].T, (8, 1)
            )
        s = np.full(ns, -1.0, dtype=np.float32)
        s[: len(slots)] = slots
        seg_a[c, :, off_ch : off_ch + nch] = (
            s.reshape(nch, 128).T.astype(BF16)
        )

    for c in range(C):
        for r in range(nb):
            b = c * nb + r
            s0, s1 = starts[b], starts[b + 1]
            m = elo_s[s0:s1]
            p = int(m.sum())
            lo_rows = srow_s[s0:s1][m].astype(np.int16)  # global rows [0, 32768)
            hi_rows = (srow_s[s0:s1][~m] - C * n_lo).astype(np.int16)
            assert p <= lo_ch[r] * 128 and (s1 - s0 - p) <= hi_ch[r] * 128
            fill_stream(idx_lo, seg_lo, c, r, lo_rows, slot_s[s0:s1][m],
                        int(lo_offs[r]), int(lo_ch[r]))
            fill_stream(idx_hi, seg_hi, c, r, hi_rows, slot_s[s0:s1][~m],
                        int(hi_offs[r]), int(hi_ch[r]))

    dis_row = np.zeros(C * nlocp, dtype=np.float32)
    dis_row[node_row] = dis

    # Precompute the one-hot selector matrices host-side (indices are static
    # across layers): S[p, c*128 + f] = (seg[p, c] == f). Streamed from DRAM
    # each layer instead of rebuilding on DVE. fp8e4 represents 0/1 exactly
    # and halves the stream bytes.
    f_iota = np.arange(128, dtype=np.float32)
    s_dtype = ml_dtypes.float8_e4m3fn if _env("GCN_SFP8", 1) else BF16

    def build_S(seg):
        s = np.asarray(seg, dtype=np.float32)  # [C, 128, tot]
        return (s[:, :, :, None] == f_iota).reshape(C, 128, -1).astype(s_dtype)

    S_lo = build_S(seg_lo)
    S_hi = build_S(seg_hi)
    return dict(
        nloc=nloc, nb=nb, nlocp=nlocp, B_LO=B_LO, n_lo=n_lo, n_hi=n_hi,
        lo_ch=lo_ch, hi_ch=hi_ch, lo_offs=lo_offs, hi_offs=hi_offs,
        lo_calls=lo_calls, hi_calls=hi_calls, tot_lo=tot_lo, tot_hi=tot_hi,
        cons_order=cons_order, node_row=node_row, dis_row=dis_row,
        idx_lo=idx_lo, idx_hi=idx_hi, seg_lo=seg_lo, seg_hi=seg_hi,
        S_lo=S_lo, S_hi=S_hi,
    )


def _build_v2(cfg: GCNConfig, plan):
    import concourse.bacc as bacc
    import concourse.tile as tile
    from concourse import mybir

    dt = mybir.dt
    F, L, C = cfg.feat, cfg.n_layers, cfg.n_cores
    nb, nlocp = plan["nb"], plan["nlocp"]
    n_lo, n_hi, B_LO = plan["n_lo"], plan["n_hi"], plan["B_LO"]
    lo_ch, hi_ch = plan["lo_ch"], plan["hi_ch"]
    lo_offs, hi_offs = plan["lo_offs"], plan["hi_offs"]
    lo_calls, hi_calls = plan["lo_calls"], plan["hi_calls"]
    tot_lo, tot_hi = plan["tot_lo"], plan["tot_hi"]
    NQ = cfg.n_swdge_queues

    nc = bacc.Bacc(
        "TRN2",
        target_bir_lowering=False,
        debug=False,
        num_devices=C,
        num_swdge_queues=NQ,
    )
    RG = [list(range(C))]

    xT_d = nc.dram_tensor("xT", [F, nlocp], dt.bfloat16, kind="ExternalInput")
    W_d = nc.dram_tensor("Wb", [L, F, F], dt.bfloat16, kind="ExternalInput")
    idxlo_d = nc.dram_tensor("idxlo", [128, tot_lo * 8], dt.int16, kind="ExternalInput")
    idxhi_d = nc.dram_tensor("idxhi", [128, tot_hi * 8], dt.int16, kind="ExternalInput")
    s_dt = dt.float8e4 if _env("GCN_SFP8", 1) else dt.bfloat16
    Slo_d = nc.dram_tensor("Slo", [128, tot_lo * 128], s_dt, kind="ExternalInput")
    Shi_d = nc.dram_tensor("Shi", [128, tot_hi * 128], s_dt, kind="ExternalInput")
    discol_d = nc.dram_tensor("discol", [128, nb], dt.float32, kind="ExternalInput")
    discol2_d = nc.dram_tensor("discol2", [128, nb], dt.float32, kind="ExternalInput")
    disrow_d = nc.dram_tensor("disrow", [1, nlocp], dt.float32, kind="ExternalInput")
    iota_d = nc.dram_tensor("iota", [128, 128], dt.bfloat16, kind="ExternalInput")
    ident_d = nc.dram_tensor("ident", [128, 128], dt.bfloat16, kind="ExternalInput")
    linw_d = nc.dram_tensor("linw", [F, 1], dt.bfloat16, kind="ExternalInput")
    out_d = nc.dram_tensor("out", [1, nlocp], dt.float32, kind="ExternalOutput")

    bounces_lo = [nc.dram_tensor(f"bnlo{i}", [n_lo, F], dt.bfloat16) for i in range(2)]
    bounces_hi = [nc.dram_tensor(f"bnhi{i}", [n_hi, F], dt.bfloat16) for i in range(2)]
    tables_lo = [
        nc.dram_tensor(f"tablo{i}", [C * n_lo, F], dt.bfloat16, addr_space="Shared")
        for i in range(2)
    ]
    tables_hi = [
        nc.dram_tensor(f"tabhi{i}", [C * n_hi, F], dt.bfloat16, addr_space="Shared")
        for i in range(2)
    ]

    with tile.TileContext(nc) as tc:
        with (
            tc.tile_pool(name="const", bufs=1) as const,
            tc.tile_pool(name="glo", bufs=4) as glo,
            tc.tile_pool(name="ghi", bufs=4) as ghi,
            tc.tile_pool(name="slo", bufs=4) as slo,
            tc.tile_pool(name="shi", bufs=4) as shi,
            tc.tile_pool(name="psA", bufs=2, space="PSUM") as psA,
            tc.tile_pool(name="psO", bufs=5, space="PSUM") as psO,
            tc.tile_pool(name="psL", bufs=1, space="PSUM") as psL,
        ):
            h0 = const.tile([F, nlocp], dt.bfloat16, tag="h0")
            h1 = const.tile([F, nlocp], dt.bfloat16, tag="h1")
            zbufs = [
                const.tile([128, nb * F], dt.bfloat16, tag=f"zbuf{i}", name=f"zbuf{i}")
                for i in range(2)
            ]
            W_sb = const.tile([F, L * F], dt.bfloat16, tag="W")
            idxlo_sb = const.tile([128, tot_lo * 8], dt.int16, tag="idxlo")
            idxhi_sb = const.tile([128, tot_hi * 8], dt.int16, tag="idxhi")
            discol = const.tile([128, nb], dt.float32, tag="discol")
            discol2 = const.tile([128, nb], dt.float32, tag="discol2")
            disrow = const.tile([1, nlocp], dt.float32, tag="disrow")
            iota = const.tile([128, 128], dt.bfloat16, tag="iota")
            ident = const.tile([128, 128], dt.bfloat16, tag="ident")
            linw = const.tile([F, 1], dt.bfloat16, tag="linw")
            orow = const.tile([1, nlocp], dt.float32, tag="orow")

            nc.sync.dma_start(out=h0[:], in_=xT_d[:])
            nc.sync.dma_start(
                out=W_sb[:].rearrange("p (l f) -> p l f", f=F),
                in_=W_d[:].rearrange("l p f -> p l f"),
            )
            nc.sync.dma_start(out=idxlo_sb[:], in_=idxlo_d[:])
            nc.sync.dma_start(out=idxhi_sb[:], in_=idxhi_d[:])
            nc.sync.dma_start(out=discol[:], in_=discol_d[:])
            nc.sync.dma_start(out=discol2[:], in_=discol2_d[:])
            nc.sync.dma_start(out=disrow[:], in_=disrow_d[:])
            nc.sync.dma_start(out=iota[:], in_=iota_d[:])
            nc.sync.dma_start(out=ident[:], in_=ident_d[:])
            nc.sync.dma_start(out=linw[:], in_=linw_d[:])

            hs = [h0, h1]
            swk = [0]

            def stage_a(l, j, h_in):
                zbuf = zbufs[l % 2]
                jsl = slice(j * 128, (j + 1) * 128)
                zp = psA.tile([128, F], dt.float32, tag="zp")
                nc.tensor.matmul(
                    out=zp[:], lhsT=h_in[:, jsl], rhs=W_sb[:, l * F : (l + 1) * F],
                    start=True, stop=True,
                )
                dc = discol if l == 0 else discol2
                nc.vector.tensor_scalar_mul(
                    out=zbuf[:, jsl], in0=zp[:], scalar1=dc[:, j : j + 1]
                )

            def bounce_ag_lo(l, part=None):
                """AllGather the lo slice. part=(b0, b1) gathers blocks
                [b0, b1); table_lo is sub-slice-major so the output region is
                contiguous: rows [C*b0*128, C*b1*128)."""
                if part is None:
                    # table_lo is sub-slice-major; a full gather is two parts
                    bounce_ag_lo(l, (0, B_LO // 2))
                    bounce_ag_lo(l, (B_LO // 2, B_LO))
                    return
                zbuf = zbufs[l % 2]
                b0, b1 = part
                # Scalar-engine DMA queue: keeps the bounce off the SP queue
                # (full of S-loads) so it fires as soon as stage A is done.
                nc.scalar.dma_start(
                    out=bounces_lo[l % 2][b0 * 128 : b1 * 128, :].rearrange(
                        "(b p) f -> p b f", p=128
                    ),
                    in_=zbuf[:, b0 * F : b1 * F].rearrange("p (b f) -> p b f", f=F),
                )
                nc.gpsimd.collective_compute(
                    "AllGather", mybir.AluOpType.bypass, replica_groups=RG,
                    ins=[bounces_lo[l % 2][b0 * 128 : b1 * 128, :]],
                    outs=[tables_lo[l % 2][C * b0 * 128 : C * b1 * 128, :]],
                )

            def bounce_ag_hi(l):
                zbuf = zbufs[l % 2]
                nc.scalar.dma_start(
                    out=bounces_hi[l % 2][:].rearrange("(b p) f -> p b f", p=128),
                    in_=zbuf[:, B_LO * F :].rearrange("p (b f) -> p b f", f=F),
                )
                nc.gpsimd.collective_compute(
                    "AllGather", mybir.AluOpType.bypass, replica_groups=RG,
                    ins=[bounces_hi[l % 2][:]], outs=[tables_hi[l % 2][:]],
                )

            cons_order = plan["cons_order"]
            HIF = _env("GCN_HIFIRST", 0)
            SPLIT_LO = _env("GCN_SPLITLO", 1) and not HIF
            half_b = B_LO // 2

            def emit_ags(l, k):
                """Issue AG pieces at the right consumption positions."""
                if HIF:
                    if k == nb - B_LO - 1:
                        bounce_ag_hi(l)
                elif SPLIT_LO:
                    if k == half_b - 1:
                        bounce_ag_lo(l, (0, half_b))
                    elif k == B_LO - 1:
                        bounce_ag_lo(l, (half_b, B_LO))
                elif k == B_LO - 1:
                    bounce_ag_lo(l)

            def emit_late_ag(l):
                if HIF:
                    bounce_ag_lo(l)
                else:
                    bounce_ag_hi(l)

            # layer 0 stage A + AG (early slice matches consumption order)
            for k, j in enumerate(cons_order):
                stage_a(0, j, h0)
                emit_ags(0, k)
            emit_late_ag(0)

            for l in range(L):
                zbuf = zbufs[l % 2]
                h_out = hs[(l + 1) % 2]
                tlo, thi = tables_lo[l % 2], tables_hi[l % 2]

                # --- issue gather calls + S builds, interleaved lo/hi
                g_tiles = {}
                s_tiles = {}

                def emit_call(kind, call):
                    ranks, coff, nchc = call
                    idx_sb = idxlo_sb if kind == 0 else idxhi_sb
                    S_d = Slo_d if kind == 0 else Shi_d
                    tab = tlo if kind == 0 else thi
                    pool = glo if kind == 0 else ghi
                    spool_ = slo if kind == 0 else shi
                    g = pool.tile([128, nchc, F], dt.bfloat16, tag=f"g{kind}")
                    nc.gpsimd.dma_gather(
                        g[:], tab[:],
                        idx_sb[:, coff * 8 : (coff + nchc) * 8],
                        nchc * 128, nchc * 128, F, elem_step=F,
                        single_packet=False,
                        queue_num=swk[0] % NQ,
                    )
                    swk[0] += 1
                    S = spool_.tile([128, nchc * 128], s_dt, tag=f"S{kind}")
                    nc.sync.dma_start(
                        out=S[:], in_=S_d[:, coff * 128 : (coff + nchc) * 128]
                    )
                    for r in ranks:
                        g_tiles[(kind, r)] = (g, S, coff)

                # merged emission: the early-AG kind's calls lead; the other
                # kind starts after DELAY calls so the in-order Pool queue
                # doesn't stall on the late AllGather too early.
                DELAY = _env("GCN_DELAY", 6 if HIF else 1)
                lead, trail = (hi_calls, lo_calls) if HIF else (lo_calls, hi_calls)
                lead_k = 1 if HIF else 0
                merged = []
                a_i, b_i = 0, 0
                while a_i < len(lead) or b_i < len(trail):
                    if a_i < len(lead) and (
                        b_i >= len(trail)
                        or a_i - DELAY < b_i * len(lead) / max(1, len(trail))
                    ):
                        merged.append((lead_k, lead[a_i]))
                        a_i += 1
                    else:
                        merged.append((1 - lead_k, trail[b_i]))
                        b_i += 1
                # Interleave call emission with consumption so the l+1
                # AllGather collectives (Pool-queued) sit mid-stream in Pool
                # order instead of behind all 70+ desc-gen calls.
                LEAD = _env("GCN_LEAD", 14)
                nci = [0]

                def emit_to(target):
                    while nci[0] < min(target, len(merged)):
                        kind, call = merged[nci[0]]
                        emit_call(kind, call)
                        nci[0] += 1

                emit_to(LEAD)

                # --- per-block consumption
                for k, j in enumerate(cons_order):
                    # all calls covering block j must be emitted; keep a lead
                    while (0, j) not in g_tiles or (1, j) not in g_tiles:
                        kind, call = merged[nci[0]]
                        emit_call(kind, call)
                        nci[0] += 1
                    emit_to(LEAD + (k + 1) * (len(merged) - LEAD) // nb)
                    jsl = slice(j * 128, (j + 1) * 128)
                    O = psO.tile([F, 128], dt.float32, tag="O")
                    nc.tensor.matmul(
                        out=O[:], lhsT=zbuf[:, jsl], rhs=ident[:], start=True,
                        stop=False,
                    )
                    korder = (
                        ((1, hi_offs, hi_ch), (0, lo_offs, lo_ch))
                        if HIF
                        else ((0, lo_offs, lo_ch), (1, hi_offs, hi_ch))
                    )
                    for kind, offs, chs in korder:
                        g, S, coff = g_tiles[(kind, j)]
                        base = int(offs[j]) - coff
                        nch_j = int(chs[j])
                        last = kind == (0 if HIF else 1)
                        for kk in range(nch_j):
                            nc.tensor.matmul(
                                out=O[:],
                                lhsT=g[:, base + kk, :],
                                rhs=S[:, (base + kk) * 128 : (base + kk + 1) * 128],
                                start=False,
                                stop=(last and kk == nch_j - 1),
                            )
                    nc.scalar.activation(
                        out=h_out[:, jsl], in_=O[:],
                        func=mybir.ActivationFunctionType.Relu,
                        scale=1.0,
                    )
                    if l + 1 < L:
                        stage_a(l + 1, j, h_out)
                        emit_ags(l + 1, k)
                emit_to(len(merged))
                if l + 1 < L:
                    emit_late_ag(l + 1)

            # --- readout: out = dis * (linw . h_L)
            h_fin = hs[L % 2]
            for k in range(0, nlocp, 512):
                w = min(512, nlocp - k)
                op = psL.tile([1, 512], dt.float32, tag="op")
                nc.tensor.matmul(
                    out=op[:, :w], lhsT=linw[:], rhs=h_fin[:, k : k + w],
                    start=True, stop=True,
                )
                nc.vector.tensor_tensor(
                    out=orow[:, k : k + w], in0=op[:, :w],
                    in1=disrow[:, k : k + w], op=mybir.AluOpType.mult,
                )
            nc.sync.dma_start(out=out_d[:], in_=orow[:])
    nc.compile()
    return nc


def _make_inputs_v2(cfg: GCNConfig, plan, x, Ws, bs, lin_w, lin_b):
    C, F, L = cfg.n_cores, cfg.feat, cfg.n_layers
    nlocp, nb = plan["nlocp"], plan["nb"]
    node_row = plan["node_row"]
    dis_row = plan["dis_row"]

    x = np.asarray(x, dtype=np.float32)
    Ws = np.asarray(Ws, dtype=np.float32)

    xrow = np.zeros((C * nlocp, F), dtype=np.float32)
    xrow[node_row] = x
    Wb = Ws.astype(BF16)
    iota = np.tile(np.arange(128, dtype=np.float32), (128, 1)).astype(BF16)
    ident = np.eye(128, dtype=np.float32).astype(BF16)
    linw = np.asarray(lin_w, dtype=np.float32).reshape(F, 1).astype(BF16)

    in_maps = []
    for c in range(C):
        rows = slice(c * nlocp, (c + 1) * nlocp)
        dloc = dis_row[rows]
        dcol = np.ascontiguousarray(dloc.reshape(nb, 128).T)
        in_maps.append(
            {
                "xT": np.ascontiguousarray(xrow[rows].T).astype(BF16),
                "Wb": Wb,
                "idxlo": np.ascontiguousarray(plan["idx_lo"][c]),
                "idxhi": np.ascontiguousarray(plan["idx_hi"][c]),
                "Slo": np.ascontiguousarray(plan["S_lo"][c]),
                "Shi": np.ascontiguousarray(plan["S_hi"][c]),
                "discol": dcol,
                "discol2": dcol * dcol,
                "disrow": dloc.reshape(1, nlocp),
                "iota": iota,
                "ident": ident,
                "linw": linw,
            }
        )
    return in_maps


def kernel(**inputs) -> np.ndarray:
    cfg = GCNConfig()
    return _kernel_impl(cfg, inputs, mode=os.environ.get("GCN_MODE", "hw"))


def _kernel_impl(cfg: GCNConfig, inputs, mode="hw", trace=False):
    x = np.asarray(inputs["x"])
    edge_index = np.asarray(inputs["edge_index"])
    use_v2 = (
        _env("GCN_V2", 1)
        and not np.any(np.asarray(inputs["bs"]))
        and not np.any(np.asarray(inputs["lin_b"]))
    )
    if use_v2:
        cfg.n_swdge_queues = _env("GCN_NSWQ", 4)
        plan = _plan_v2(cfg, edge_index)
        nc = _build_v2(cfg, plan)
        in_maps = _make_inputs_v2(
            cfg, plan, x, inputs["Ws"], inputs["bs"], inputs["lin_w"], inputs["lin_b"]
        )
    else:
        plan = _plan(cfg, edge_index)
        nc = _build(cfg, plan)
        in_maps = _make_inputs(
            cfg, plan, x, inputs["Ws"], inputs["bs"], inputs["lin_w"], inputs["lin_b"]
        )
    if mode == "sim":
        from concourse import bass_interp

        sim = bass_interp.MultiCoreSim(nc, cfg.n_cores)
        for c in range(cfg.n_cores):
            for k, v in in_maps[c].items():
                sim.cores[c].tensor(k)[:] = v
        sim.simulate()
        outs = [
            {"out": np.asarray(sim.cores[c].mem_tensor("out"))}
            for c in range(cfg.n_cores)
        ]
        result = _reassemble(cfg, plan, outs)
        return result.astype(np.float32)
    else:
        from concourse.bass_utils import run_bass_kernel_spmd

        res = run_bass_kernel_spmd(
            nc, in_maps, core_ids=list(range(cfg.n_cores)), trace=trace
        )
        out = _reassemble(cfg, plan, res.results)
        if trace:
            return out.astype(np.float32), res
        return out.astype(np.float32)


if __name__ == "__main__":
    pass

